# revision 12
# baseline (speedup 1.0000x reference)
"""Causal self-attention (B=2, T=2048, C=1024, H=16) on 8 trn2 NeuronCores.

Sharding: core c = (batch b = c // 4, head-group g = c % 4). Each core
computes, for its batch, QKV for heads [4g, 4g+4), causal attention, and a
partial output projection through rows [256g, 256g+256) of W_proj. The host
sums the 4 bf16 partial projections per batch and adds b_proj.

v3 structure (PE-bound; all matmul inputs bf16, fp32 PSUM):
  - Scores computed TRANSPOSED (S^T[k, q]) so exp(S^T) is directly the P^T
    operand of the PV matmul. Two heads per score step run CONCURRENTLY in
    the PE array via row-group packing (K=64 at array rows 0/64).
  - Diagonal k-blocks are column-trimmed (only q >= 128m is computed), and
    the remaining partial triangle is masked ON THE PE: an identity-weight
    matmul accumulates a constant triangular -20000 into the score PSUM
    before exp, which then underflows to exact 0. No vector/gpsimd masking.
  - V carries 64 appended ones columns (lhsT [128, 128]), so the PV matmul
    leaves the softmax denominator REPLICATED across PSUM rows 64:127 - the
    reciprocal runs wide on [64, 1024] (both heads, all DVE lanes) and the
    normalize multiplies read PSUM directly and write yT (bf16). The whole
    softmax tail is PE + one DVE op + two DVE muls; no DMA, no gpsimd.
  - QKV / V / projection matmul groups are emitted as fillers BETWEEN
    attention steps; x chunks are prefetched two chunks ahead; weight DMAs
    are ordered so the first QKV group's operands arrive first, with ~36
    N=128 warm-up matmuls keeping the PE HAM clock-gate at K=8/8 during
    the initial DMA wait.
  - Partial projection outputs are written bf16 (halves output DMA).
"""

import sys
from collections import deque

for _p in ("/opt/trn_rl_repo",):
    if _p not in sys.path:
        sys.path.insert(0, _p)

import numpy as np
import ml_dtypes

import concourse.bass as bass
import concourse.tile as tile
from concourse import bacc, mybir
from concourse.bass_utils import run_bass_kernel_spmd

BF16 = mybir.dt.bfloat16
F32 = mybir.dt.float32
NP_BF16 = ml_dtypes.bfloat16

B, T, C = 2, 2048, 1024
H, D = 16, 64
N_CORES = 8
CT = C // 128   # 8 contraction tiles
TQ = T // 128   # 16 key blocks
QC = T // 512   # 4 query chunks
SCALE = 1.0 / np.sqrt(D)
DEPTH = 2       # score-stage lookahead ahead of PV consumes

_compiled = None


def _build_nc():
    nc = bacc.Bacc("TRN2", target_bir_lowering=False, debug=False,
                   enable_asserts=False)

    xT_d = nc.dram_tensor("xT", [QC, C, 512], BF16, kind="ExternalInput")
    wqk_d = nc.dram_tensor("wqk", [4, C, 128], BF16, kind="ExternalInput")
    wv_d = nc.dram_tensor("wv", [C, 256], BF16, kind="ExternalInput")
    wp_d = nc.dram_tensor("wp", [256, C], BF16, kind="ExternalInput")
    bqk_d = nc.dram_tensor("bqk", [128, 4], F32, kind="ExternalInput")
    bv_d = nc.dram_tensor("bv", [128, 256], BF16, kind="ExternalInput")
    ident_d = nc.dram_tensor("ident", [128, 128], BF16, kind="ExternalInput")
    ninf_d = nc.dram_tensor("ninf", [128, 128], BF16, kind="ExternalInput")
    out_d = nc.dram_tensor("out", [T, C], BF16, kind="ExternalOutput")

    Exp = mybir.ActivationFunctionType.Exp

    with tile.TileContext(nc) as tc:
        with (
            tc.tile_pool(name="const", bufs=1) as cpool,
            tc.tile_pool(name="qkT", bufs=1) as qkpool,
            tc.tile_pool(name="vbuf", bufs=1) as vpool,
            tc.tile_pool(name="ybuf", bufs=1) as ypool,
            tc.tile_pool(name="pt", bufs=10) as ptpool,
            tc.tile_pool(name="norm", bufs=2) as npool,
            tc.tile_pool(name="ostage", bufs=3) as opool,
            tc.tile_pool(name="mmps", bufs=2, space="PSUM") as mmps,
            tc.tile_pool(name="sps", bufs=4, space="PSUM") as sps,
            tc.tile_pool(name="yps", bufs=1, space="PSUM") as yps,
        ):
            # ---- SBUF residents ----
            xT_s = cpool.tile([128, CT, T], BF16)
            wqk_s = cpool.tile([128, CT, 512], BF16)
            wv_s = cpool.tile([128, CT, 256], BF16)
            wp_s = cpool.tile([128, 2, C], BF16)
            bqk_s = cpool.tile([128, 4], F32)
            bv_s = cpool.tile([128, 256], BF16)
            ident_s = cpool.tile([128, 128], BF16)
            ninf_s = cpool.tile([128, 128], BF16)
            warm = cpool.tile([128, 128], BF16)
            warma = cpool.tile([128, 1], F32)

            qkT_s = qkpool.tile([128, 4, T], BF16)
            v_s = vpool.tile([128, TQ, 4, 128], BF16)
            yT_s = ypool.tile([128, 2, T], BF16)

            # ---- DMA order: first-needed first. One multi-dim DMA per
            # tensor: the input queue is descriptor-rate-bound (~600ns per
            # instruction regardless of size), so batching matters.
            def dma_wqk(j):
                nc.sync.dma_start(
                    out=wqk_s[:, :, 128 * j:128 * (j + 1)],
                    in_=wqk_d.ap()[j].rearrange("(i p) c -> p i c", p=128))

            def dma_chunk(t4):
                nc.sync.dma_start(
                    out=xT_s[:, :, 512 * t4:512 * (t4 + 1)],
                    in_=xT_d.ap()[t4].rearrange("(i p) q -> p i q", p=128))

            # x chunk 0 rides the scalar-engine HWDGE queue in parallel with
            # the weight stream on the sync queue (ACT is idle at startup).
            nc.scalar.dma_start(
                out=xT_s[:, :, 0:512],
                in_=xT_d.ap()[0].rearrange("(i p) q -> p i q", p=128))
            nc.sync.dma_start(out=bqk_s[:], in_=bqk_d.ap()[:])
            nc.sync.dma_start(out=ident_s[:], in_=ident_d.ap()[:])
            nc.sync.dma_start(out=ninf_s[:], in_=ninf_d.ap()[:])
            dma_wqk(0)
            nc.sync.dma_start(
                out=wv_s[:], in_=wv_d.ap().rearrange("(i p) c -> p i c", p=128))
            nc.sync.dma_start(out=bv_s[:], in_=bv_d.ap()[:])
            dma_wqk(2)
            dma_wqk(1)
            dma_wqk(3)
            dma_chunk(1)
            nc.sync.dma_start(
                out=wp_s[:], in_=wp_d.ap().rearrange("(n p) c -> p n c", p=128))

            # ---- PE + ACT warm-up (runs under the initial DMA wait) ----
            nc.vector.memset(warm[:], 0.0)
            nc.vector.memset(warma[:], 0.0)
            nc.scalar.activation(warma[:], warma[:], Exp)
            wps = mmps.tile([128, 512], F32, tag="mm")
            for _ in range(48):
                nc.tensor.matmul(wps[:, 0:128], warm[:], warm[:],
                                 start=True, stop=True)

            nc.vector.memset(v_s[:, :, :, 64:128], 1.0)

            # ---- work groups (emitted directly or as fillers) ----
            def qkv_group(j, t4):
                # j 0: Q heads {0,1}; 1: Q {2,3}; 2: K {0,1}; 3: K {2,3}
                ps = mmps.tile([128, 512], F32, tag="mm")
                for i in range(CT):
                    nc.tensor.matmul(
                        ps[:],
                        wqk_s[:, i, 128 * j:128 * (j + 1)],
                        xT_s[:, i, 512 * t4:512 * (t4 + 1)],
                        start=(i == 0), stop=(i == CT - 1),
                    )
                nc.vector.tensor_scalar_add(
                    qkT_s[:, j, 512 * t4:512 * (t4 + 1)], ps[:], bqk_s[:, j:j + 1])

            def v_group(t):
                ps = mmps.tile([128, 256], F32, tag="mm")
                for i in range(CT):
                    nc.tensor.matmul(
                        ps[:],
                        xT_s[:, i, 128 * t:128 * (t + 1)],
                        wv_s[:, i, :],
                        start=(i == 0), stop=(i == CT - 1),
                    )
                nc.vector.tensor_add(
                    v_s[:, t, :, 0:64],
                    ps[:].rearrange("p (h d) -> p h d", h=4),
                    bv_s[:].rearrange("p (h d) -> p h d", h=4))

            def proj_group(t):
                o_t = opool.tile([128, C], BF16, tag="o")
                for n in range(2):
                    ps = mmps.tile([128, 512], F32, tag="mm")
                    for p2 in range(2):
                        nc.tensor.matmul(
                            ps[:],
                            yT_s[:, p2, 128 * t:128 * (t + 1)],
                            wp_s[:, p2, 512 * n:512 * (n + 1)],
                            start=(p2 == 0), stop=(p2 == 1),
                        )
                    nc.vector.tensor_copy(o_t[:, 512 * n:512 * (n + 1)], ps[:])
                nc.sync.dma_start(out=out_d.ap()[128 * t:128 * (t + 1), :], in_=o_t[:])

            fillers = deque()

            def emit_filler(n=1):
                for _ in range(n):
                    if fillers:
                        fillers.popleft()()

            # ---- prologue compute: chunk-0 pair-0 operands ----
            qkv_group(0, 0)
            qkv_group(2, 0)
            for t in range(4):
                v_group(t)

            # ---- per-chunk filler plans ----
            def plan(qc):
                fs = []
                if qc == 0:
                    fs += [lambda: qkv_group(1, 0), lambda: qkv_group(3, 0),
                           lambda: qkv_group(0, 1), lambda: qkv_group(2, 1)]
                    fs += [lambda t=t: v_group(t) for t in range(4, 8)]
                elif qc == 1:
                    fs += [lambda: qkv_group(1, 1), lambda: qkv_group(3, 1),
                           lambda: dma_chunk(2)]
                    fs += [lambda t=t: proj_group(t) for t in range(0, 4)]
                    fs += [lambda: qkv_group(0, 2), lambda: qkv_group(2, 2)]
                    fs += [lambda t=t: v_group(t) for t in range(8, 12)]
                elif qc == 2:
                    fs += [lambda: qkv_group(1, 2), lambda: qkv_group(3, 2),
                           lambda: dma_chunk(3)]
                    fs += [lambda t=t: proj_group(t) for t in range(4, 8)]
                    fs += [lambda: qkv_group(0, 3), lambda: qkv_group(2, 3)]
                    fs += [lambda t=t: v_group(t) for t in range(12, 16)]
                else:
                    fs += [lambda: qkv_group(1, 3), lambda: qkv_group(3, 3)]
                    fs += [lambda t=t: proj_group(t) for t in range(8, 10)]
                fillers.extend(fs)

            # ---- attention ----
            def stage(kb, qc, jq, jk, pts):
                m = kb - 4 * qc
                off = 128 * m if m > 0 else 0
                ptA = ptpool.tile([128, 512], BF16, tag="pt")
                ptB = ptpool.tile([128, 512], BF16, tag="pt")
                prs = []
                for hi, pt in ((0, ptA), (1, ptB)):
                    part = slice(64 * hi, 64 * (hi + 1))
                    s_ps = sps.tile([128, 512], F32, tag="s")
                    nc.tensor.matmul(
                        s_ps[:, off:512],
                        qkT_s[part, jk, 128 * kb:128 * (kb + 1)],
                        qkT_s[part, jq, 512 * qc + off:512 * (qc + 1)],
                        start=True, stop=(m < 0),
                        tile_position=(64 * hi, 0), skip_group_check=True)
                    prs.append((s_ps, pt))
                if m >= 0:
                    # accumulate triangular -20000 into the diag block; exp
                    # of masked entries then underflows to exact 0.
                    for s_ps, _ in prs:
                        nc.tensor.matmul(
                            s_ps[:, off:off + 128], ident_s[:], ninf_s[:],
                            start=False, stop=True, skip_group_check=True)
                for s_ps, pt in prs:
                    nc.scalar.activation(pt[:, off:512], s_ps[:, off:512],
                                         Exp, scale=SCALE)
                pts[kb] = (ptA, ptB, off)

            def consume(kb, p, y_ps, nkb, pts):
                ptA, ptB, off = pts.pop(kb)
                for hi, pt in ((0, ptA), (1, ptB)):
                    nc.tensor.matmul(
                        y_ps[:, 512 * hi + off:512 * (hi + 1)],
                        v_s[:, kb, 2 * p + hi, :],
                        pt[:, off:512],
                        start=(kb == 0), stop=(kb == nkb - 1))

            fin_ps = []
            for qc in range(QC):
                plan(qc)
                for p in range(2):
                    jq, jk = p, 2 + p
                    nkb = 4 * qc + 4
                    y_ps = yps.tile([128, 1024], F32, tag="y")
                    pts = {}
                    for kb in range(min(DEPTH, nkb)):
                        stage(kb, qc, jq, jk, pts)
                    # batch-2: longer same-geometry PE runs (score quads,
                    # then PV quads) so LDWEIGHTS pipelines better.
                    for base in range(0, nkb, 2):
                        for kb in (base + DEPTH, base + DEPTH + 1):
                            if kb < nkb:
                                stage(kb, qc, jq, jk, pts)
                        consume(base, p, y_ps, nkb, pts)
                        consume(base + 1, p, y_ps, nkb, pts)
                        emit_filler(2)

                    # normalize: PSUM rows 64:127 all hold the denominator
                    # (ones-block PV trick) - one wide reciprocal serves
                    # both heads. The denominator is staged through SBUF on
                    # ACT (parallel to DVE) because the custom-DVE
                    # reciprocal's bitwise seed reads PSUM wrong on HW
                    # (sim models PSUM as plain f32).
                    dd = npool.tile([64, 1024], F32, tag="dd")
                    nc.scalar.copy(dd[:], y_ps[64:128, :])
                    rb = npool.tile([64, 1024], F32, tag="rb")
                    nc.vector.reciprocal_approx_fast(rb[:], dd[:])
                    last = (qc == QC - 1 and p == 1)
                    if last:
                        # cover the final norm chain: two full projection
                        # groups plus the pair-0 halves of the last four.
                        proj_group(10)
                        proj_group(11)
                        for t in range(12, 16):
                            ps = sps.tile([128, 512], F32, tag="s")
                            nc.tensor.matmul(
                                ps[:], yT_s[:, 0, 128 * t:128 * (t + 1)],
                                wp_s[:, 0, 0:512], start=True, stop=False,
                                skip_group_check=True)
                            fin_ps.append(ps)
                    else:
                        emit_filler(3)
                    for hi in range(2):
                        nc.vector.tensor_mul(
                            yT_s[64 * hi:64 * (hi + 1), p, 512 * qc:512 * (qc + 1)],
                            y_ps[0:64, 512 * hi:512 * hi + 512],
                            rb[:, 512 * hi:512 * hi + 512])

            # ---- epilogue: finish the split projection groups ----
            emit_filler(len(fillers))
            for t in range(12, 16):
                o_t = opool.tile([128, C], BF16, tag="o")
                ps = fin_ps[t - 12]
                nc.tensor.matmul(
                    ps[:], yT_s[:, 1, 128 * t:128 * (t + 1)],
                    wp_s[:, 1, 0:512], start=False, stop=True,
                    skip_group_check=True)
                nc.vector.tensor_copy(o_t[:, 0:512], ps[:])
                ps2 = mmps.tile([128, 512], F32, tag="mm")
                for p2 in range(2):
                    nc.tensor.matmul(
                        ps2[:], yT_s[:, p2, 128 * t:128 * (t + 1)],
                        wp_s[:, p2, 512:1024], start=(p2 == 0), stop=(p2 == 1))
                nc.vector.tensor_copy(o_t[:, 512:1024], ps2[:])
                nc.sync.dma_start(out=out_d.ap()[128 * t:128 * (t + 1), :], in_=o_t[:])

    nc.compile()
    return nc


def _shard_inputs(x, W_attn, b_attn, W_proj, b_proj):
    """Build the 8 per-core input maps (numpy, bf16 where applicable)."""
    # ninf: triangular mask addend for a diagonal k-block: S^T entry (p, j)
    # is masked where p > j -> add -20000 there (exp underflows to 0).
    pp = np.arange(128)[:, None]
    jj = np.arange(128)[None, :]
    ninf = np.where(pp > jj, -20000.0, 0.0).astype(NP_BF16)
    ident = np.eye(128, dtype=NP_BF16)
    in_maps = []
    for c in range(N_CORES):
        b, g = c // 4, c % 4
        ch = slice(256 * g, 256 * (g + 1))
        wq = W_attn[:, ch]
        wk = W_attn[:, C:][:, ch]
        wv = W_attn[:, 2 * C:][:, ch]
        # j-major: [4, C, 128] so j-slices stream independently
        wqk = np.stack([wq[:, 0:128], wq[:, 128:256],
                        wk[:, 0:128], wk[:, 128:256]], axis=0).astype(NP_BF16)
        bq = b_attn[ch]
        bk = b_attn[C:][ch]
        bv = b_attn[2 * C:][ch]
        bqk = np.concatenate([bq, bk]).reshape(4, 128).T.astype(np.float32)  # [128, 4]
        xTc = np.ascontiguousarray(
            x[b].T.reshape(C, QC, 512).transpose(1, 0, 2)).astype(NP_BF16)
        in_maps.append({
            "xT": xTc,
            "wqk": np.ascontiguousarray(wqk),
            "wv": wv.astype(NP_BF16),
            "wp": W_proj[ch, :].astype(NP_BF16),
            "bqk": np.ascontiguousarray(bqk),
            "bv": np.broadcast_to(bv.astype(NP_BF16), (128, 256)).copy(),
            "ident": ident,
            "ninf": ninf,
        })
    return in_maps


def _run(in_maps, trace=False, **kw):
    global _compiled
    if _compiled is None:
        _compiled = _build_nc()
    return run_bass_kernel_spmd(_compiled, in_maps, list(range(N_CORES)),
                                trace=trace, **kw)


def kernel(x, W_attn, b_attn, W_proj, b_proj):
    x = np.asarray(x, dtype=np.float32)
    W_attn = np.asarray(W_attn, dtype=np.float32)
    b_attn = np.asarray(b_attn, dtype=np.float32)
    W_proj = np.asarray(W_proj, dtype=np.float32)
    b_proj = np.asarray(b_proj, dtype=np.float32)

    in_maps = _shard_inputs(x, W_attn, b_attn, W_proj, b_proj)
    res = _run(in_maps)
    out = np.zeros((B, T, C), dtype=np.float32)
    for c in range(N_CORES):
        out[c // 4] += np.asarray(res.results[c]["out"], dtype=np.float32)
    out += b_proj
    return out


# revision 13
# speedup vs baseline: 1.0204x; 1.0204x over previous
"""Causal self-attention (B=2, T=2048, C=1024, H=16) on 8 trn2 NeuronCores.

Sharding: core c = (batch b = c // 4, head-group g = c % 4). Each core
computes, for its batch, QKV for heads [4g, 4g+4), causal attention, and a
partial output projection through rows [256g, 256g+256) of W_proj. The host
sums the 4 bf16 partial projections per batch and adds b_proj.

v3 structure (PE-bound; all matmul inputs bf16, fp32 PSUM):
  - Scores computed TRANSPOSED (S^T[k, q]) so exp(S^T) is directly the P^T
    operand of the PV matmul. Two heads per score step run CONCURRENTLY in
    the PE array via row-group packing (K=64 at array rows 0/64).
  - Diagonal k-blocks are column-trimmed (only q >= 128m is computed), and
    the remaining partial triangle is masked ON THE PE: an identity-weight
    matmul accumulates a constant triangular -20000 into the score PSUM
    before exp, which then underflows to exact 0. No vector/gpsimd masking.
  - V carries 64 appended ones columns (lhsT [128, 128]), so the PV matmul
    leaves the softmax denominator REPLICATED across PSUM rows 64:127 - the
    reciprocal runs wide on [64, 1024] (both heads, all DVE lanes) and the
    normalize multiplies read PSUM directly and write yT (bf16). The whole
    softmax tail is PE + one DVE op + two DVE muls; no DMA, no gpsimd.
  - QKV / V / projection matmul groups are emitted as fillers BETWEEN
    attention steps; x chunks are prefetched two chunks ahead; weight DMAs
    are ordered so the first QKV group's operands arrive first, with ~36
    N=128 warm-up matmuls keeping the PE HAM clock-gate at K=8/8 during
    the initial DMA wait.
  - Partial projection outputs are written bf16 (halves output DMA).
"""

import sys
from collections import deque

for _p in ("/opt/trn_rl_repo",):
    if _p not in sys.path:
        sys.path.insert(0, _p)

import numpy as np
import ml_dtypes

import concourse.bass as bass
import concourse.tile as tile
from concourse import bacc, mybir
from concourse.bass_utils import run_bass_kernel_spmd

BF16 = mybir.dt.bfloat16
F32 = mybir.dt.float32
NP_BF16 = ml_dtypes.bfloat16

B, T, C = 2, 2048, 1024
H, D = 16, 64
N_CORES = 8
CT = C // 128   # 8 contraction tiles
TQ = T // 128   # 16 key blocks
QC = T // 512   # 4 query chunks
SCALE = 1.0 / np.sqrt(D)
DEPTH = 2       # score-stage lookahead ahead of PV consumes

_compiled = None


def _build_nc():
    nc = bacc.Bacc("TRN2", target_bir_lowering=False, debug=False,
                   enable_asserts=False)

    xT_d = nc.dram_tensor("xT", [QC, C, 512], BF16, kind="ExternalInput")
    wqk_d = nc.dram_tensor("wqk", [4, C, 128], BF16, kind="ExternalInput")
    wv_d = nc.dram_tensor("wv", [C, 256], BF16, kind="ExternalInput")
    wp_d = nc.dram_tensor("wp", [256, C], BF16, kind="ExternalInput")
    bqk_d = nc.dram_tensor("bqk", [128, 4], F32, kind="ExternalInput")
    bv_d = nc.dram_tensor("bv", [128, 256], BF16, kind="ExternalInput")
    ident_d = nc.dram_tensor("ident", [128, 128], BF16, kind="ExternalInput")
    ninf_d = nc.dram_tensor("ninf", [128, 128], BF16, kind="ExternalInput")
    out_d = nc.dram_tensor("out", [T, C], BF16, kind="ExternalOutput")

    Exp = mybir.ActivationFunctionType.Exp

    with tile.TileContext(nc) as tc:
        with (
            tc.tile_pool(name="const", bufs=1) as cpool,
            tc.tile_pool(name="qkT", bufs=1) as qkpool,
            tc.tile_pool(name="vbuf", bufs=1) as vpool,
            tc.tile_pool(name="ybuf", bufs=1) as ypool,
            tc.tile_pool(name="pt", bufs=10) as ptpool,
            tc.tile_pool(name="norm", bufs=2) as npool,
            tc.tile_pool(name="ostage", bufs=3) as opool,
            tc.tile_pool(name="mmps", bufs=2, space="PSUM") as mmps,
            tc.tile_pool(name="sps", bufs=4, space="PSUM") as sps,
            tc.tile_pool(name="yps", bufs=1, space="PSUM") as yps,
        ):
            # ---- SBUF residents ----
            xT_s = cpool.tile([128, CT, T], BF16)
            wqk_s = cpool.tile([128, CT, 512], BF16)
            wv_s = cpool.tile([128, CT, 256], BF16)
            wp_s = cpool.tile([128, 2, C], BF16)
            bqk_s = cpool.tile([128, 4], F32)
            bv_s = cpool.tile([128, 256], BF16)
            ident_s = cpool.tile([128, 128], BF16)
            ninf_s = cpool.tile([128, 128], BF16)
            warm = cpool.tile([128, 128], BF16)
            warma = cpool.tile([128, 1], F32)

            qkT_s = qkpool.tile([128, 4, T], BF16)
            v_s = vpool.tile([128, TQ, 4, 128], BF16)
            yT_s = ypool.tile([128, 2, T], BF16)

            # ---- DMA order: first-needed first. One multi-dim DMA per
            # tensor: the input queue is descriptor-rate-bound (~600ns per
            # instruction regardless of size), so batching matters.
            def dma_wqk(j):
                nc.sync.dma_start(
                    out=wqk_s[:, :, 128 * j:128 * (j + 1)],
                    in_=wqk_d.ap()[j].rearrange("(i p) c -> p i c", p=128))

            def dma_chunk(t4):
                nc.sync.dma_start(
                    out=xT_s[:, :, 512 * t4:512 * (t4 + 1)],
                    in_=xT_d.ap()[t4].rearrange("(i p) q -> p i q", p=128))

            # x chunk 0 rides the scalar-engine HWDGE queue in parallel with
            # the weight stream on the sync queue (ACT is idle at startup).
            nc.scalar.dma_start(
                out=xT_s[:, :, 0:512],
                in_=xT_d.ap()[0].rearrange("(i p) q -> p i q", p=128))
            nc.sync.dma_start(out=bqk_s[:], in_=bqk_d.ap()[:])
            nc.sync.dma_start(out=ident_s[:], in_=ident_d.ap()[:])
            nc.sync.dma_start(out=ninf_s[:], in_=ninf_d.ap()[:])
            dma_wqk(0)
            nc.sync.dma_start(
                out=wv_s[:], in_=wv_d.ap().rearrange("(i p) c -> p i c", p=128))
            nc.sync.dma_start(out=bv_s[:], in_=bv_d.ap()[:])
            dma_wqk(2)
            dma_wqk(1)
            dma_wqk(3)
            dma_chunk(1)
            nc.sync.dma_start(
                out=wp_s[:], in_=wp_d.ap().rearrange("(n p) c -> p n c", p=128))

            # ---- PE + ACT warm-up (runs under the initial DMA wait) ----
            nc.vector.memset(warm[:], 0.0)
            nc.vector.memset(warma[:], 0.0)
            nc.scalar.activation(warma[:], warma[:], Exp)
            wps = mmps.tile([128, 512], F32, tag="mm")
            for _ in range(48):
                nc.tensor.matmul(wps[:, 0:128], warm[:], warm[:],
                                 start=True, stop=True)

            nc.vector.memset(v_s[:, :, :, 64:128], 1.0)

            # ---- work groups (emitted directly or as fillers) ----
            def qkv_group(j, t4):
                # j 0: Q heads {0,1}; 1: Q {2,3}; 2: K {0,1}; 3: K {2,3}
                ps = mmps.tile([128, 512], F32, tag="mm")
                for i in range(CT):
                    nc.tensor.matmul(
                        ps[:],
                        wqk_s[:, i, 128 * j:128 * (j + 1)],
                        xT_s[:, i, 512 * t4:512 * (t4 + 1)],
                        start=(i == 0), stop=(i == CT - 1),
                    )
                nc.vector.tensor_scalar_add(
                    qkT_s[:, j, 512 * t4:512 * (t4 + 1)], ps[:], bqk_s[:, j:j + 1])

            def v_group(t):
                ps = mmps.tile([128, 256], F32, tag="mm")
                for i in range(CT):
                    nc.tensor.matmul(
                        ps[:],
                        xT_s[:, i, 128 * t:128 * (t + 1)],
                        wv_s[:, i, :],
                        start=(i == 0), stop=(i == CT - 1),
                    )
                nc.vector.tensor_add(
                    v_s[:, t, :, 0:64],
                    ps[:].rearrange("p (h d) -> p h d", h=4),
                    bv_s[:].rearrange("p (h d) -> p h d", h=4))

            def proj_group(t):
                o_t = opool.tile([128, C], BF16, tag="o")
                for n in range(2):
                    ps = mmps.tile([128, 512], F32, tag="mm")
                    for p2 in range(2):
                        nc.tensor.matmul(
                            ps[:],
                            yT_s[:, p2, 128 * t:128 * (t + 1)],
                            wp_s[:, p2, 512 * n:512 * (n + 1)],
                            start=(p2 == 0), stop=(p2 == 1),
                        )
                    nc.vector.tensor_copy(o_t[:, 512 * n:512 * (n + 1)], ps[:])
                nc.sync.dma_start(out=out_d.ap()[128 * t:128 * (t + 1), :], in_=o_t[:])

            fillers = deque()

            def emit_filler(n=1):
                for _ in range(n):
                    if fillers:
                        fillers.popleft()()

            # ---- prologue compute: chunk-0 pair-0 operands ----
            qkv_group(0, 0)
            qkv_group(2, 0)
            for t in range(4):
                v_group(t)

            # ---- per-chunk filler plans ----
            def plan(qc):
                fs = []
                if qc == 0:
                    fs += [lambda: qkv_group(1, 0), lambda: qkv_group(3, 0),
                           lambda: qkv_group(0, 1), lambda: qkv_group(2, 1)]
                    fs += [lambda t=t: v_group(t) for t in range(4, 8)]
                elif qc == 1:
                    fs += [lambda: qkv_group(1, 1), lambda: qkv_group(3, 1),
                           lambda: dma_chunk(2)]
                    fs += [lambda t=t: proj_group(t) for t in range(0, 4)]
                    fs += [lambda: qkv_group(0, 2), lambda: qkv_group(2, 2)]
                    fs += [lambda t=t: v_group(t) for t in range(8, 12)]
                elif qc == 2:
                    fs += [lambda: qkv_group(1, 2), lambda: qkv_group(3, 2),
                           lambda: dma_chunk(3)]
                    fs += [lambda t=t: proj_group(t) for t in range(4, 8)]
                    fs += [lambda: qkv_group(0, 3), lambda: qkv_group(2, 3)]
                    fs += [lambda t=t: v_group(t) for t in range(12, 16)]
                else:
                    fs += [lambda: qkv_group(1, 3), lambda: qkv_group(3, 3)]
                    fs += [lambda t=t: proj_group(t) for t in range(8, 10)]
                fillers.extend(fs)

            # ---- attention ----
            def stage(kb, qc, jq, jk, pts):
                m = kb - 4 * qc
                off = 128 * m if m > 0 else 0
                ptA = ptpool.tile([128, 512], BF16, tag="pt")
                ptB = ptpool.tile([128, 512], BF16, tag="pt")
                prs = []
                for hi, pt in ((0, ptA), (1, ptB)):
                    part = slice(64 * hi, 64 * (hi + 1))
                    s_ps = sps.tile([128, 512], F32, tag="s")
                    nc.tensor.matmul(
                        s_ps[:, off:512],
                        qkT_s[part, jk, 128 * kb:128 * (kb + 1)],
                        qkT_s[part, jq, 512 * qc + off:512 * (qc + 1)],
                        start=True, stop=(m < 0),
                        tile_position=(64 * hi, 0), skip_group_check=True)
                    prs.append((s_ps, pt))
                if m >= 0:
                    # accumulate triangular -20000 into the diag block; exp
                    # of masked entries then underflows to exact 0.
                    for s_ps, _ in prs:
                        nc.tensor.matmul(
                            s_ps[:, off:off + 128], ident_s[:], ninf_s[:],
                            start=False, stop=True, skip_group_check=True)
                for s_ps, pt in prs:
                    nc.scalar.activation(pt[:, off:512], s_ps[:, off:512],
                                         Exp, scale=SCALE)
                pts[kb] = (ptA, ptB, off)

            def consume(kb, p, y_ps, nkb, pts):
                ptA, ptB, off = pts.pop(kb)
                for hi, pt in ((0, ptA), (1, ptB)):
                    nc.tensor.matmul(
                        y_ps[:, 512 * hi + off:512 * (hi + 1)],
                        v_s[:, kb, 2 * p + hi, :],
                        pt[:, off:512],
                        start=(kb == 0), stop=(kb == nkb - 1))

            fin_ps = []
            for qc in range(QC):
                plan(qc)
                for p in range(2):
                    jq, jk = p, 2 + p
                    nkb = 4 * qc + 4
                    y_ps = yps.tile([128, 1024], F32, tag="y")
                    pts = {}
                    for kb in range(min(DEPTH, nkb)):
                        stage(kb, qc, jq, jk, pts)
                    for kb in range(nkb):
                        if kb + DEPTH < nkb:
                            stage(kb + DEPTH, qc, jq, jk, pts)
                        consume(kb, p, y_ps, nkb, pts)
                        emit_filler(1)

                    # normalize: PSUM rows 64:127 all hold the denominator
                    # (ones-block PV trick) - one wide reciprocal serves
                    # both heads. The denominator is staged through SBUF on
                    # ACT (parallel to DVE) because the custom-DVE
                    # reciprocal's bitwise seed reads PSUM wrong on HW
                    # (sim models PSUM as plain f32).
                    dd = npool.tile([64, 1024], F32, tag="dd")
                    nc.scalar.copy(dd[:], y_ps[64:128, :])
                    rb = npool.tile([64, 1024], F32, tag="rb")
                    nc.vector.reciprocal_approx_fast(rb[:], dd[:])
                    last = (qc == QC - 1 and p == 1)
                    if last:
                        # cover the final norm chain: two full projection
                        # groups plus the pair-0 halves of the last four.
                        proj_group(10)
                        proj_group(11)
                        for t in range(12, 16):
                            ps = sps.tile([128, 512], F32, tag="s")
                            nc.tensor.matmul(
                                ps[:], yT_s[:, 0, 128 * t:128 * (t + 1)],
                                wp_s[:, 0, 0:512], start=True, stop=False,
                                skip_group_check=True)
                            fin_ps.append(ps)
                    else:
                        emit_filler(3)
                    for hi in range(2):
                        nc.vector.tensor_mul(
                            yT_s[64 * hi:64 * (hi + 1), p, 512 * qc:512 * (qc + 1)],
                            y_ps[0:64, 512 * hi:512 * hi + 512],
                            rb[:, 512 * hi:512 * hi + 512])

            # ---- epilogue: finish the split projection groups ----
            emit_filler(len(fillers))
            for t in range(12, 16):
                o_t = opool.tile([128, C], BF16, tag="o")
                ps = fin_ps[t - 12]
                nc.tensor.matmul(
                    ps[:], yT_s[:, 1, 128 * t:128 * (t + 1)],
                    wp_s[:, 1, 0:512], start=False, stop=True,
                    skip_group_check=True)
                nc.vector.tensor_copy(o_t[:, 0:512], ps[:])
                ps2 = mmps.tile([128, 512], F32, tag="mm")
                for p2 in range(2):
                    nc.tensor.matmul(
                        ps2[:], yT_s[:, p2, 128 * t:128 * (t + 1)],
                        wp_s[:, p2, 512:1024], start=(p2 == 0), stop=(p2 == 1))
                nc.vector.tensor_copy(o_t[:, 512:1024], ps2[:])
                nc.sync.dma_start(out=out_d.ap()[128 * t:128 * (t + 1), :], in_=o_t[:])

    nc.compile()
    return nc


def _shard_inputs(x, W_attn, b_attn, W_proj, b_proj):
    """Build the 8 per-core input maps (numpy, bf16 where applicable)."""
    # ninf: triangular mask addend for a diagonal k-block: S^T entry (p, j)
    # is masked where p > j -> add -20000 there (exp underflows to 0).
    pp = np.arange(128)[:, None]
    jj = np.arange(128)[None, :]
    ninf = np.where(pp > jj, -20000.0, 0.0).astype(NP_BF16)
    ident = np.eye(128, dtype=NP_BF16)
    in_maps = []
    for c in range(N_CORES):
        b, g = c // 4, c % 4
        ch = slice(256 * g, 256 * (g + 1))
        wq = W_attn[:, ch]
        wk = W_attn[:, C:][:, ch]
        wv = W_attn[:, 2 * C:][:, ch]
        # j-major: [4, C, 128] so j-slices stream independently
        wqk = np.stack([wq[:, 0:128], wq[:, 128:256],
                        wk[:, 0:128], wk[:, 128:256]], axis=0).astype(NP_BF16)
        bq = b_attn[ch]
        bk = b_attn[C:][ch]
        bv = b_attn[2 * C:][ch]
        bqk = np.concatenate([bq, bk]).reshape(4, 128).T.astype(np.float32)  # [128, 4]
        xTc = np.ascontiguousarray(
            x[b].T.reshape(C, QC, 512).transpose(1, 0, 2)).astype(NP_BF16)
        in_maps.append({
            "xT": xTc,
            "wqk": np.ascontiguousarray(wqk),
            "wv": wv.astype(NP_BF16),
            "wp": W_proj[ch, :].astype(NP_BF16),
            "bqk": np.ascontiguousarray(bqk),
            "bv": np.broadcast_to(bv.astype(NP_BF16), (128, 256)).copy(),
            "ident": ident,
            "ninf": ninf,
        })
    return in_maps


def _run(in_maps, trace=False, **kw):
    global _compiled
    if _compiled is None:
        _compiled = _build_nc()
    return run_bass_kernel_spmd(_compiled, in_maps, list(range(N_CORES)),
                                trace=trace, **kw)


def kernel(x, W_attn, b_attn, W_proj, b_proj):
    x = np.asarray(x, dtype=np.float32)
    W_attn = np.asarray(W_attn, dtype=np.float32)
    b_attn = np.asarray(b_attn, dtype=np.float32)
    W_proj = np.asarray(W_proj, dtype=np.float32)
    b_proj = np.asarray(b_proj, dtype=np.float32)

    in_maps = _shard_inputs(x, W_attn, b_attn, W_proj, b_proj)
    res = _run(in_maps)
    out = np.zeros((B, T, C), dtype=np.float32)
    for c in range(N_CORES):
        out[c // 4] += np.asarray(res.results[c]["out"], dtype=np.float32)
    out += b_proj
    return out


# revision 19
# speedup vs baseline: 1.1346x; 1.1120x over previous
"""Causal self-attention (B=2, T=2048, C=1024, H=16) on 8 trn2 NeuronCores.

Sharding: core c = (batch b = c // 4, head-group g = c % 4). Each core
computes, for its batch, QKV for heads [4g, 4g+4), causal attention, and a
partial output projection through rows [256g, 256g+256) of W_proj. The host
sums the 4 bf16 partial projections per batch and adds b_proj.

v3 structure (PE-bound; all matmul inputs bf16, fp32 PSUM):
  - Scores computed TRANSPOSED (S^T[k, q]) so exp(S^T) is directly the P^T
    operand of the PV matmul. Two heads per score step run CONCURRENTLY in
    the PE array via row-group packing (K=64 at array rows 0/64).
  - Diagonal k-blocks are column-trimmed (only q >= 128m is computed), and
    the remaining partial triangle is masked ON THE PE: an identity-weight
    matmul accumulates a constant triangular -20000 into the score PSUM
    before exp, which then underflows to exact 0. No vector/gpsimd masking.
  - V carries 64 appended ones columns (lhsT [128, 128]), so the PV matmul
    leaves the softmax denominator REPLICATED across PSUM rows 64:127 - the
    reciprocal runs wide on [64, 1024] (both heads, all DVE lanes) and the
    normalize multiplies read PSUM directly and write yT (bf16). The whole
    softmax tail is PE + one DVE op + two DVE muls; no DMA, no gpsimd.
  - QKV / V / projection matmul groups are emitted as fillers BETWEEN
    attention steps; x chunks are prefetched two chunks ahead; weight DMAs
    are ordered so the first QKV group's operands arrive first, with ~36
    N=128 warm-up matmuls keeping the PE HAM clock-gate at K=8/8 during
    the initial DMA wait.
  - Partial projection outputs are written bf16 (halves output DMA).
"""

import sys
from collections import deque

for _p in ("/opt/trn_rl_repo",):
    if _p not in sys.path:
        sys.path.insert(0, _p)

import numpy as np
import ml_dtypes

import concourse.bass as bass
import concourse.tile as tile
from concourse import bacc, mybir
from concourse.bass_utils import run_bass_kernel_spmd

BF16 = mybir.dt.bfloat16
F32 = mybir.dt.float32
NP_BF16 = ml_dtypes.bfloat16

B, T, C = 2, 2048, 1024
H, D = 16, 64
N_CORES = 8
CT = C // 128   # 8 contraction tiles
TQ = T // 128   # 16 key blocks
QC = T // 512   # 4 query chunks
SCALE = 1.0 / np.sqrt(D)
DEPTH = 2       # score-stage lookahead ahead of PV consumes

_compiled = None


def _build_nc():
    nc = bacc.Bacc("TRN2", target_bir_lowering=False, debug=False,
                   enable_asserts=False)

    xT_d = nc.dram_tensor("xT", [QC, C, 512], BF16, kind="ExternalInput")
    wqk_d = nc.dram_tensor("wqk", [4, C, 128], BF16, kind="ExternalInput")
    wv_d = nc.dram_tensor("wv", [C, 256], BF16, kind="ExternalInput")
    wp_d = nc.dram_tensor("wp", [256, C], BF16, kind="ExternalInput")
    bqk_d = nc.dram_tensor("bqk", [128, 4], F32, kind="ExternalInput")
    bv_d = nc.dram_tensor("bv", [128, 256], BF16, kind="ExternalInput")
    ident_d = nc.dram_tensor("ident", [128, 128], BF16, kind="ExternalInput")
    ninf_d = nc.dram_tensor("ninf", [128, 128], BF16, kind="ExternalInput")
    out_d = nc.dram_tensor("out", [T, C], BF16, kind="ExternalOutput")

    Exp = mybir.ActivationFunctionType.Exp

    with tile.TileContext(nc) as tc:
        with (
            tc.tile_pool(name="const", bufs=1) as cpool,
            tc.tile_pool(name="qkT", bufs=1) as qkpool,
            tc.tile_pool(name="vbuf", bufs=1) as vpool,
            tc.tile_pool(name="ybuf", bufs=1) as ypool,
            tc.tile_pool(name="pt", bufs=5) as ptpool,
            tc.tile_pool(name="norm", bufs=2) as npool,
            tc.tile_pool(name="ostage", bufs=3) as opool,
            tc.tile_pool(name="mmps", bufs=2, space="PSUM") as mmps,
            tc.tile_pool(name="sps", bufs=2, space="PSUM") as sps,
            tc.tile_pool(name="yps", bufs=1, space="PSUM") as yps,
        ):
            # ---- SBUF residents ----
            xT_s = cpool.tile([128, CT, T], BF16)
            wqk_s = cpool.tile([128, CT, 512], BF16)
            wv_s = cpool.tile([128, CT, 256], BF16)
            wp_s = cpool.tile([128, 2, C], BF16)
            bqk_s = cpool.tile([128, 4], F32)
            bv_s = cpool.tile([128, 256], BF16)
            ident_s = cpool.tile([128, 128], BF16)
            ninf_s = cpool.tile([128, 128], BF16)
            warm = cpool.tile([128, 128], BF16)
            warma = cpool.tile([128, 1], F32)

            qkT_s = qkpool.tile([128, 4, T], BF16)
            v_s = vpool.tile([128, TQ, 4, 128], BF16)
            yT_s = ypool.tile([128, 2, T], BF16)

            # ---- DMA order: first-needed first. One multi-dim DMA per
            # tensor: the input queue is descriptor-rate-bound (~600ns per
            # instruction regardless of size), so batching matters.
            def dma_wqk(j):
                nc.sync.dma_start(
                    out=wqk_s[:, :, 128 * j:128 * (j + 1)],
                    in_=wqk_d.ap()[j].rearrange("(i p) c -> p i c", p=128))

            def dma_chunk(t4):
                nc.sync.dma_start(
                    out=xT_s[:, :, 512 * t4:512 * (t4 + 1)],
                    in_=xT_d.ap()[t4].rearrange("(i p) q -> p i q", p=128))

            # x chunk 0 rides the scalar-engine HWDGE queue in parallel with
            # the weight stream on the sync queue (ACT is idle at startup).
            nc.scalar.dma_start(
                out=xT_s[:, :, 0:512],
                in_=xT_d.ap()[0].rearrange("(i p) q -> p i q", p=128))
            nc.sync.dma_start(out=bqk_s[:], in_=bqk_d.ap()[:])
            nc.sync.dma_start(out=ident_s[:], in_=ident_d.ap()[:])
            nc.sync.dma_start(out=ninf_s[:], in_=ninf_d.ap()[:])
            dma_wqk(0)
            nc.sync.dma_start(
                out=wv_s[:], in_=wv_d.ap().rearrange("(i p) c -> p i c", p=128))
            nc.sync.dma_start(out=bv_s[:], in_=bv_d.ap()[:])
            dma_wqk(2)
            dma_wqk(1)
            dma_wqk(3)
            dma_chunk(1)
            nc.sync.dma_start(
                out=wp_s[:], in_=wp_d.ap().rearrange("(n p) c -> p n c", p=128))

            # ---- PE + ACT warm-up (runs under the initial DMA wait) ----
            nc.vector.memset(warm[:], 0.0)
            nc.vector.memset(warma[:], 0.0)
            nc.scalar.activation(warma[:], warma[:], Exp)
            wps = mmps.tile([128, 512], F32, tag="mm")
            for _ in range(48):
                nc.tensor.matmul(wps[:, 0:128], warm[:], warm[:],
                                 start=True, stop=True)

            nc.vector.memset(v_s[:, :, :, 64:128], 1.0)

            # ---- work groups (emitted directly or as fillers) ----
            def qkv_group(j, t4):
                # j 0: Q heads {0,1}; 1: Q {2,3}; 2: K {0,1}; 3: K {2,3}
                ps = mmps.tile([128, 512], F32, tag="mm")
                for i in range(CT):
                    nc.tensor.matmul(
                        ps[:],
                        wqk_s[:, i, 128 * j:128 * (j + 1)],
                        xT_s[:, i, 512 * t4:512 * (t4 + 1)],
                        start=(i == 0), stop=(i == CT - 1),
                    )
                nc.vector.tensor_scalar_add(
                    qkT_s[:, j, 512 * t4:512 * (t4 + 1)], ps[:], bqk_s[:, j:j + 1])

            def v_group(t):
                ps = mmps.tile([128, 256], F32, tag="mm")
                for i in range(CT):
                    nc.tensor.matmul(
                        ps[:],
                        xT_s[:, i, 128 * t:128 * (t + 1)],
                        wv_s[:, i, :],
                        start=(i == 0), stop=(i == CT - 1),
                    )
                nc.vector.tensor_add(
                    v_s[:, t, :, 0:64],
                    ps[:].rearrange("p (h d) -> p h d", h=4),
                    bv_s[:].rearrange("p (h d) -> p h d", h=4))

            def proj_group(t):
                o_t = opool.tile([128, C], BF16, tag="o")
                for n in range(2):
                    ps = mmps.tile([128, 512], F32, tag="mm")
                    for p2 in range(2):
                        nc.tensor.matmul(
                            ps[:],
                            yT_s[:, p2, 128 * t:128 * (t + 1)],
                            wp_s[:, p2, 512 * n:512 * (n + 1)],
                            start=(p2 == 0), stop=(p2 == 1),
                        )
                    nc.vector.tensor_copy(o_t[:, 512 * n:512 * (n + 1)], ps[:])
                nc.sync.dma_start(out=out_d.ap()[128 * t:128 * (t + 1), :], in_=o_t[:])

            fillers = deque()

            def emit_filler(n=1):
                for _ in range(n):
                    if fillers:
                        fillers.popleft()()

            # ---- prologue compute: chunk-0 pair-0 operands ----
            qkv_group(0, 0)
            qkv_group(2, 0)
            for t in range(4):
                v_group(t)

            # ---- per-chunk filler plans ----
            def plan(qc):
                fs = []
                if qc == 0:
                    fs += [lambda: qkv_group(1, 0), lambda: qkv_group(3, 0),
                           lambda: qkv_group(0, 1), lambda: qkv_group(2, 1)]
                    fs += [lambda t=t: v_group(t) for t in range(4, 8)]
                elif qc == 1:
                    fs += [lambda: qkv_group(1, 1), lambda: qkv_group(3, 1),
                           lambda: dma_chunk(2)]
                    fs += [lambda t=t: proj_group(t) for t in range(0, 4)]
                    fs += [lambda: qkv_group(0, 2), lambda: qkv_group(2, 2)]
                    fs += [lambda t=t: v_group(t) for t in range(8, 12)]
                elif qc == 2:
                    fs += [lambda: qkv_group(1, 2), lambda: qkv_group(3, 2),
                           lambda: dma_chunk(3)]
                    fs += [lambda t=t: proj_group(t) for t in range(4, 8)]
                    fs += [lambda: qkv_group(0, 3), lambda: qkv_group(2, 3)]
                    fs += [lambda t=t: v_group(t) for t in range(12, 16)]
                else:
                    fs += [lambda: qkv_group(1, 3), lambda: qkv_group(3, 3)]
                    fs += [lambda t=t: proj_group(t) for t in range(8, 10)]
                fillers.extend(fs)

            # ---- attention ----
            def stage(kb, qc, jq, jk, pts):
                m = kb - 4 * qc
                off = 128 * m if m > 0 else 0
                s_ps = sps.tile([128, 2, 512], F32, tag="s")
                pt = ptpool.tile([128, 2, 512], BF16, tag="pt")
                for hi in range(2):
                    part = slice(64 * hi, 64 * (hi + 1))
                    nc.tensor.matmul(
                        s_ps[:, hi, off:512],
                        qkT_s[part, jk, 128 * kb:128 * (kb + 1)],
                        qkT_s[part, jq, 512 * qc + off:512 * (qc + 1)],
                        start=True, stop=(m < 0),
                        tile_position=(64 * hi, 0), skip_group_check=True)
                if m >= 0:
                    # accumulate triangular -20000 into the diag block; exp
                    # of masked entries then underflows to exact 0.
                    for hi in range(2):
                        nc.tensor.matmul(
                            s_ps[:, hi, off:off + 128], ident_s[:], ninf_s[:],
                            start=False, stop=True, skip_group_check=True)
                # ONE exp for both heads: ACTIVATE pays a 352-cycle fixed
                # cost per op, so fusing halves the overhead.
                nc.scalar.activation(pt[:, :, off:512], s_ps[:, :, off:512],
                                     Exp, scale=SCALE)
                pts[kb] = (pt, off)

            def consume(kb, p, y_ps, nkb, pts):
                pt, off = pts.pop(kb)
                for hi in range(2):
                    nc.tensor.matmul(
                        y_ps[:, 512 * hi + off:512 * (hi + 1)],
                        v_s[:, kb, 2 * p + hi, :],
                        pt[:, hi, off:512],
                        start=(kb == 0), stop=(kb == nkb - 1))

            fin_ps = []
            for qc in range(QC):
                plan(qc)
                for p in range(2):
                    jq, jk = p, 2 + p
                    nkb = 4 * qc + 4
                    y_ps = yps.tile([128, 1024], F32, tag="y")
                    pts = {}
                    for kb in range(min(DEPTH, nkb)):
                        stage(kb, qc, jq, jk, pts)
                    for kb in range(nkb):
                        if kb + DEPTH < nkb:
                            stage(kb + DEPTH, qc, jq, jk, pts)
                        consume(kb, p, y_ps, nkb, pts)
                        emit_filler(1)

                    # normalize: PSUM rows 64:127 all hold the denominator
                    # (ones-block PV trick) - one wide reciprocal serves
                    # both heads. The denominator is staged through SBUF on
                    # ACT (parallel to DVE) because the custom-DVE
                    # reciprocal's bitwise seed reads PSUM wrong on HW
                    # (sim models PSUM as plain f32).
                    dd = npool.tile([64, 1024], F32, tag="dd")
                    nc.vector.tensor_copy(dd[:], y_ps[64:128, :])
                    rb = npool.tile([64, 1024], F32, tag="rb")
                    nc.vector.reciprocal_approx_fast(rb[:], dd[:])
                    last = (qc == QC - 1 and p == 1)
                    if last:
                        # cover the final norm chain: two full projection
                        # groups plus the pair-0 halves of two more.
                        proj_group(10)
                        proj_group(11)
                        for t in range(12, 14):
                            ps = sps.tile([128, 512], F32, tag="s")
                            nc.tensor.matmul(
                                ps[:], yT_s[:, 0, 128 * t:128 * (t + 1)],
                                wp_s[:, 0, 0:512], start=True, stop=False,
                                skip_group_check=True)
                            fin_ps.append(ps)
                    else:
                        emit_filler(3)
                    for hi in range(2):
                        nc.vector.tensor_mul(
                            yT_s[64 * hi:64 * (hi + 1), p, 512 * qc:512 * (qc + 1)],
                            y_ps[0:64, 512 * hi:512 * hi + 512],
                            rb[:, 512 * hi:512 * hi + 512])

            # ---- epilogue: finish the split projection groups ----
            emit_filler(len(fillers))
            for t in range(12, 14):
                o_t = opool.tile([128, C], BF16, tag="o")
                ps = fin_ps[t - 12]
                nc.tensor.matmul(
                    ps[:], yT_s[:, 1, 128 * t:128 * (t + 1)],
                    wp_s[:, 1, 0:512], start=False, stop=True,
                    skip_group_check=True)
                nc.vector.tensor_copy(o_t[:, 0:512], ps[:])
                ps2 = mmps.tile([128, 512], F32, tag="mm")
                for p2 in range(2):
                    nc.tensor.matmul(
                        ps2[:], yT_s[:, p2, 128 * t:128 * (t + 1)],
                        wp_s[:, p2, 512:1024], start=(p2 == 0), stop=(p2 == 1))
                nc.vector.tensor_copy(o_t[:, 512:1024], ps2[:])
                nc.sync.dma_start(out=out_d.ap()[128 * t:128 * (t + 1), :], in_=o_t[:])
            for t in range(14, 16):
                proj_group(t)

    nc.compile()
    return nc


def _shard_inputs(x, W_attn, b_attn, W_proj, b_proj):
    """Build the 8 per-core input maps (numpy, bf16 where applicable)."""
    # ninf: triangular mask addend for a diagonal k-block: S^T entry (p, j)
    # is masked where p > j -> add -20000 there (exp underflows to 0).
    pp = np.arange(128)[:, None]
    jj = np.arange(128)[None, :]
    ninf = np.where(pp > jj, -20000.0, 0.0).astype(NP_BF16)
    ident = np.eye(128, dtype=NP_BF16)
    in_maps = []
    for c in range(N_CORES):
        b, g = c // 4, c % 4
        ch = slice(256 * g, 256 * (g + 1))
        wq = W_attn[:, ch]
        wk = W_attn[:, C:][:, ch]
        wv = W_attn[:, 2 * C:][:, ch]
        # j-major: [4, C, 128] so j-slices stream independently
        wqk = np.stack([wq[:, 0:128], wq[:, 128:256],
                        wk[:, 0:128], wk[:, 128:256]], axis=0).astype(NP_BF16)
        bq = b_attn[ch]
        bk = b_attn[C:][ch]
        bv = b_attn[2 * C:][ch]
        bqk = np.concatenate([bq, bk]).reshape(4, 128).T.astype(np.float32)  # [128, 4]
        xTc = np.ascontiguousarray(
            x[b].T.reshape(C, QC, 512).transpose(1, 0, 2)).astype(NP_BF16)
        in_maps.append({
            "xT": xTc,
            "wqk": np.ascontiguousarray(wqk),
            "wv": wv.astype(NP_BF16),
            "wp": W_proj[ch, :].astype(NP_BF16),
            "bqk": np.ascontiguousarray(bqk),
            "bv": np.broadcast_to(bv.astype(NP_BF16), (128, 256)).copy(),
            "ident": ident,
            "ninf": ninf,
        })
    return in_maps


def _run(in_maps, trace=False, **kw):
    global _compiled
    if _compiled is None:
        _compiled = _build_nc()
    return run_bass_kernel_spmd(_compiled, in_maps, list(range(N_CORES)),
                                trace=trace, **kw)


def kernel(x, W_attn, b_attn, W_proj, b_proj):
    x = np.asarray(x, dtype=np.float32)
    W_attn = np.asarray(W_attn, dtype=np.float32)
    b_attn = np.asarray(b_attn, dtype=np.float32)
    W_proj = np.asarray(W_proj, dtype=np.float32)
    b_proj = np.asarray(b_proj, dtype=np.float32)

    in_maps = _shard_inputs(x, W_attn, b_attn, W_proj, b_proj)
    res = _run(in_maps)
    out = np.zeros((B, T, C), dtype=np.float32)
    for c in range(N_CORES):
        out[c // 4] += np.asarray(res.results[c]["out"], dtype=np.float32)
    out += b_proj
    return out
